# revision 25
# baseline (speedup 1.0000x reference)
"""Trainium2 Bass kernel for nn_AudioModel (LSTM(40->64) -> last-h -> MLP head).

Strategy (8 NeuronCores, pure data parallel):
  - Each core processes a 1024-row batch shard; no collectives.
  - Device layout is fully transposed ("gate-major"): hidden units live on
    SBUF partitions, batch on the free dim. The per-core batch of 1024 is
    split into two halves A/B stacked on partitions (rows 0-63 = half A,
    rows 64-127 = half B) so every elementwise op runs on all 128 lanes.
  - Matmuls are block-diagonal over the A/B halves:
      gates_X[128, 512] = wx_X[82, 128].T @ ft_t[82, 512]   (x-side, + bias)
                        + wh_X[128, 128].T @ h_prev[128, 512] (h-side)
    where ft_t carries both halves' features plus constant-1 rows (bias).
  - All nonlinearities are reparameterized to sigmoid only:
      tanh(x) = 2*sigmoid(2x) - 1, folded into weight scaling and fused
      scalar_tensor_tensor ops; h is tracked as h' = h/2.
    So ScalarE does one [128, 2048] sigmoid over all 4 gates + one
    [128, 512] sigmoid for tanh(c) per step.
  - feats are transposed on the host into [T, 82, 512] per-core tiles so all
    device DMAs are clean contiguous loads.
"""

import os
import sys
from contextlib import ExitStack

import ml_dtypes
import numpy as np
_BF = np.dtype(np.float16)

for _p in ("/opt/trn_rl_repo",):
    if _p not in sys.path:
        sys.path.insert(0, _p)

import concourse.bass as bass
import concourse.mybir as mybir
from concourse import bacc
from concourse.bass_utils import run_bass_kernel_spmd
from concourse.tile import TileContext


def _install_ntff_hook():
    """Provide antenv.axon_hooks if the image lacks it, so trace=True works."""
    try:
        import antenv.axon_hooks  # noqa: F401

        return
    except ImportError:
        pass
    import contextlib
    import ctypes
    import types

    so_path = "/opt/axon/libaxon_pjrt.so"
    hook = None
    if os.path.exists(so_path):
        try:
            lib = ctypes.CDLL(so_path)
            if hasattr(lib, "axon_start_nrt_profile"):
                lib.axon_start_nrt_profile.argtypes = [
                    ctypes.POINTER(ctypes.c_int64),
                    ctypes.c_size_t,
                ]
                lib.axon_start_nrt_profile.restype = ctypes.c_int64
                lib.axon_stop_nrt_profile.argtypes = [ctypes.c_char_p]
                lib.axon_stop_nrt_profile.restype = ctypes.c_int64

                @contextlib.contextmanager
                def _hook(output_dir, device_ids):
                    import jax

                    jax.devices()
                    if device_ids:
                        ids = (ctypes.c_int64 * len(device_ids))(*device_ids)
                        rc = lib.axon_start_nrt_profile(ids, len(device_ids))
                    else:
                        rc = lib.axon_start_nrt_profile(None, 0)
                    if rc != 0:
                        raise RuntimeError(f"axon_start_nrt_profile rc={rc}")
                    try:
                        yield
                    finally:
                        n = lib.axon_stop_nrt_profile(str(output_dir).encode())
                        print(f"profile: {n} file(s) written to {output_dir}", file=sys.stderr)

                hook = _hook
        except OSError:
            hook = None

    mod = types.ModuleType("antenv.axon_hooks")
    mod._hook = hook
    mod.get_axon_ntff_profile_hook = lambda: mod._hook
    mod.set_axon_ntff_profile_hook = lambda h: setattr(mod, "_hook", h)
    sys.modules["antenv.axon_hooks"] = mod


_install_ntff_hook()


def _enable_ldw_opt():
    """walrus is invoked with --enable-ldw-opt=false; our inner loop reloads
    identical PE weights for back-to-back stream matmuls, so dedup helps."""
    from concourse import bass_utils as _bu

    if getattr(_bu, "_ldw_patch", False):
        return
    _orig = _bu.run_command

    def _patched(cmd, *a, **kw):
        return _orig(cmd, *a, **kw)

    _bu.run_command = _patched
    _bu._ldw_patch = True


# --enable-ldw-opt=true breaks walrus codegen (visitInstLdweights); keep off.

F32 = mybir.dt.float32
F32R = mybir.dt.float32r
FH = mybir.dt.float16
AF = mybir.ActivationFunctionType
OP = mybir.AluOpType

B, T, NI, H = 8192, 100, 40, 64
NCORES = 8
BL = B // NCORES  # 1024 rows per core
HB = BL // 2  # 512 = half-batch (free dim of all tiles)
KX = 2 * (NI + 1)  # 82 = A feats(40) + ones(1) + B feats(40) + ones(1)

LAST_RESULT = None
_NC_CACHE = {}


def _build_nc():
    nc = bacc.Bacc("TRN2", target_bir_lowering=False, debug=False)

    ft = nc.dram_tensor("ft", [T, KX, HB], FH, kind="ExternalInput")
    wx = nc.dram_tensor("wx", [KX, 512], FH, kind="ExternalInput")
    wh = nc.dram_tensor("wh", [128, 512], FH, kind="ExternalInput")
    w1 = nc.dram_tensor("w1", [128, 64], FH, kind="ExternalInput")
    b1 = nc.dram_tensor("b1", [64, 1], F32, kind="ExternalInput")
    w2 = nc.dram_tensor("w2", [64, 2], FH, kind="ExternalInput")
    b2 = nc.dram_tensor("b2", [2, 1], F32, kind="ExternalInput")
    out = nc.dram_tensor("out", [2, HB], F32, kind="ExternalOutput")

    with TileContext(nc) as tc, ExitStack() as ctx:
        const = ctx.enter_context(tc.tile_pool(name="const", bufs=1))
        ftp = ctx.enter_context(tc.tile_pool(name="ftp", bufs=12))
        gp = ctx.enter_context(tc.tile_pool(name="gp", bufs=1, space="PSUM"))
        sp = ctx.enter_context(tc.tile_pool(name="sp", bufs=3))
        dp = ctx.enter_context(tc.tile_pool(name="dp", bufs=4))
        hp = ctx.enter_context(tc.tile_pool(name="hp", bufs=3))

        wx_s = const.tile([KX, 512], FH)
        nc.sync.dma_start(wx_s[:], wx[:, :])
        wh_s = const.tile([128, 512], FH)
        nc.sync.dma_start(wh_s[:], wh[:, :])
        w1_s = const.tile([128, 64], FH)
        nc.sync.dma_start(w1_s[:], w1[:, :])
        b1_s = const.tile([64, 1], F32)
        nc.sync.dma_start(b1_s[:], b1[:, :])
        w2_s = const.tile([64, 2], FH)
        nc.sync.dma_start(w2_s[:], w2[:, :])
        b2_s = const.tile([2, 1], F32)
        nc.sync.dma_start(b2_s[:], b2[:, :])

        c2 = const.tile([128, HB], FH)  # cell state (fp32, in-place)
        h_final = const.tile([128, HB], FH)  # last step's h' for the head

        # Two phase-shifted streams over the free dim (cols 0:256 / 256:512)
        # so PE / ScalarE / VectorE overlap across the serial recurrence.
        NS = 2
        SW = HB // NS  # 256
        h_prev = [None] * NS  # h' = h/2; h0 == 0 so step 0 skips h-matmuls

        ft_cache = {}
        for t in range(T):
            if t not in ft_cache:
                ft2 = ftp.tile([KX, 2 * HB], FH, name=f"ft2_{t}", tag="ft2")
                nc.sync.dma_start(
                    ft2[:, :].rearrange("p (u c) -> p u c", c=HB),
                    ft[t : t + 2].rearrange("u p c -> p u c"),
                )
                ft_cache[t] = ft2[:, 0:HB]
                ft_cache[t + 1] = ft2[:, HB : 2 * HB]
            ft_t = ft_cache.pop(t)

            # x-side matmuls for both streams first (no h dependency; adjacent
            # same-weight pairs dedupe their LDWEIGHTS under --enable-ldw-opt),
            # then per-stream h-side matmuls on the critical chain.
            # one full psum bank per gate per stream (8 banks total, bufs=1)
            # so accumulation groups never share a bank and x-side matmuls can
            # run ahead of the h-dependency without clearing sibling gates.
            gates_t = []
            for s in range(NS):
                gates_t.append(gp.tile([128, 4 * 512], F32, tag=f"g{s}", name=f"g{s}_{t}"))
            for X in range(4):
                for s in range(NS):
                    cs = slice(SW * s, SW * (s + 1))
                    nc.tensor.matmul(
                        gates_t[s][:, 512 * X : 512 * X + SW],
                        wx_s[:, 128 * X : 128 * (X + 1)],
                        ft_t[:, cs],
                        start=True,
                        stop=(h_prev[s] is None),
                    )
            for s in range(NS):
                if h_prev[s] is None:
                    continue
                for X in range(4):
                    nc.tensor.matmul(
                        gates_t[s][:, 512 * X : 512 * X + SW],
                        wh_s[:, 128 * X : 128 * (X + 1)],
                        h_prev[s],
                        start=False,
                        stop=True,
                    )

            for s in range(NS):
                cs = slice(SW * s, SW * (s + 1))
                gates = gates_t[s]
                S = sp.tile([128, 4 * SW], FH, tag=f"S{s}")
                # one sigmoid over all 4 banks; bank g holds sig(2*a_g)
                gv = gates[:, :].rearrange("p (g c) -> p g c", c=512)[:, :, 0:SW]
                sv = S[:, :].rearrange("p (g c) -> p g c", c=SW)
                nc.scalar.activation(sv, gv, AF.Sigmoid)
                sig_i = S[:, 0 * SW : 1 * SW]
                sig_f = S[:, 1 * SW : 2 * SW]
                sig_o = S[:, 2 * SW : 3 * SW]
                sig_g = S[:, 3 * SW : 4 * SW]
                c2s = c2[:, cs]

                # c2 holds c/2:  c/2 = (sig(2g)-0.5)*i + f*(c/2)_prev
                if t == 0:
                    nc.vector.scalar_tensor_tensor(c2s, sig_g, -0.5, sig_i, OP.add, OP.mult)
                else:
                    t1 = dp.tile([128, SW], FH, tag=f"t1{s}")
                    nc.vector.scalar_tensor_tensor(t1[:], sig_g, -0.5, sig_i, OP.add, OP.mult)
                    fm = dp.tile([128, SW], FH, tag=f"fm{s}")
                    nc.vector.tensor_mul(fm[:], sig_f, c2s)
                    nc.vector.tensor_add(c2s, t1[:], fm[:])
                # scv = tanh(2 * c/2) = tanh(c)
                scv = dp.tile([128, SW], FH, tag=f"scv{s}")
                nc.scalar.activation(scv[:], c2s, AF.Tanh, scale=2.0)
                # h = o * tanh(c)
                if t == T - 1:
                    h_new = h_final[:, cs]
                else:
                    h_new = hp.tile([128, SW], FH, name=f"hn{s}_{t}", tag=f"h{s}")[:]
                nc.vector.tensor_mul(h_new, scv[:], sig_o)
                h_prev[s] = h_new

        # classifier head: relu(2*W1 @ h' + b1) then W2 @ . + b2
        hid_ps = gp.tile([64, HB], F32, tag="g0")
        nc.tensor.matmul(hid_ps[:], w1_s[:], h_final[:], start=True, stop=True)
        hr = dp.tile([64, HB], FH, tag="hr")
        nc.scalar.activation(hr[:], hid_ps[:], AF.Relu, bias=b1_s[:])
        sc_ps = gp.tile([2, HB], F32, tag="g1")
        nc.tensor.matmul(sc_ps[:], w2_s[:], hr[:], start=True, stop=True)
        ov = dp.tile([2, HB], F32, tag="ov")
        nc.scalar.activation(ov[:], sc_ps[:], AF.Identity, bias=b2_s[:])
        nc.sync.dma_start(out[:, :], ov[:])

    nc.compile()
    return nc


def _get_nc():
    if "nc" not in _NC_CACHE:
        _NC_CACHE["nc"] = _build_nc()
    return _NC_CACHE["nc"]


def _prep_weights(inputs):
    W_ih = np.asarray(inputs["W_ih"], np.float32)  # [256, 40], gate order i,f,g,o
    W_hh = np.asarray(inputs["W_hh"], np.float32)  # [256, 64]
    bias = (np.asarray(inputs["b_ih"], np.float32) + np.asarray(inputs["b_hh"], np.float32))
    W1 = np.asarray(inputs["W1"], np.float32)  # [32, 64]
    b1 = np.asarray(inputs["b1"], np.float32)  # [32]
    W2 = np.asarray(inputs["W2"], np.float32)  # [1, 32]
    b2 = np.asarray(inputs["b2"], np.float32)  # [1]

    # device gate-bank order [i, f, o, g]; bank g carries 2x scale (sig(2x) trick)
    gate_order = [0, 1, 3, 2]
    gate_scale = [1.0, 1.0, 1.0, 2.0]
    wx = np.zeros((KX, 512), _BF)
    wh = np.zeros((128, 512), _BF)
    for X, gsel in enumerate(gate_order):
        sc = gate_scale[X]
        Wxe = (sc * W_ih[64 * gsel : 64 * (gsel + 1)]).astype(np.float32)  # [64, 40]
        Whe = (sc * W_hh[64 * gsel : 64 * (gsel + 1)]).astype(np.float32)  # [64, 64]
        be = (sc * bias[64 * gsel : 64 * (gsel + 1)]).astype(np.float32)  # [64]
        wx[0:NI, 128 * X : 128 * X + 64] = Wxe.T
        wx[NI, 128 * X : 128 * X + 64] = be
        wx[NI + 1 : 2 * NI + 1, 128 * X + 64 : 128 * X + 128] = Wxe.T
        wx[2 * NI + 1, 128 * X + 64 : 128 * X + 128] = be
        wh[0:64, 128 * X : 128 * X + 64] = Whe.T
        wh[64:128, 128 * X + 64 : 128 * X + 128] = Whe.T

    w1 = np.zeros((128, 64), _BF)
    w1[0:64, 0:32] = W1.T
    w1[64:128, 32:64] = W1.T
    b1v = np.concatenate([b1, b1]).reshape(64, 1).astype(np.float32)
    w2m = np.zeros((64, 2), _BF)
    w2m[0:32, 0] = W2[0]
    w2m[32:64, 1] = W2[0]
    b2v = np.array([[b2[0]], [b2[0]]], np.float32)
    return wx, wh, w1, b1v, w2m, b2v


def kernel(**inputs):
    global LAST_RESULT
    feats = np.asarray(inputs["feats"], np.float32)
    wx, wh, w1m, b1v, w2m, b2v = _prep_weights(inputs)

    in_maps = []
    for c in range(NCORES):
        shard = feats[c * BL : (c + 1) * BL]  # [1024, 100, 40]
        x = np.ascontiguousarray(shard.transpose(1, 2, 0))  # [100, 40, 1024]
        ftc = np.empty((T, KX, HB), _BF)
        ftc[:, 0:NI, :] = x[:, :, 0:HB]
        ftc[:, NI, :] = 1.0
        ftc[:, NI + 1 : 2 * NI + 1, :] = x[:, :, HB:]
        ftc[:, 2 * NI + 1, :] = 1.0
        in_maps.append(
            {"ft": ftc, "wx": wx, "wh": wh, "w1": w1m, "b1": b1v, "w2": w2m, "b2": b2v}
        )

    nc = _get_nc()
    trace = bool(os.environ.get("KERNEL_TRACE"))
    res = run_bass_kernel_spmd(nc, in_maps, core_ids=list(range(NCORES)), trace=trace)
    LAST_RESULT = res

    outs = np.empty((B, 1), np.float32)
    for c in range(NCORES):
        o = np.asarray(res.results[c]["out"])  # [2, 512]
        outs[c * BL : c * BL + HB, 0] = o[0]
        outs[c * BL + HB : (c + 1) * BL, 0] = o[1]
    return outs


if __name__ == "__main__":
    rng = np.random.default_rng(0)
    fake = {
        "feats": rng.standard_normal((B, T, NI), dtype=np.float32),
        "W_ih": rng.standard_normal((256, NI), dtype=np.float32) * 0.1,
        "W_hh": rng.standard_normal((256, H), dtype=np.float32) * 0.1,
        "b_ih": rng.standard_normal(256, dtype=np.float32) * 0.1,
        "b_hh": rng.standard_normal(256, dtype=np.float32) * 0.1,
        "W1": rng.standard_normal((32, H), dtype=np.float32) * 0.1,
        "b1": np.zeros(32, np.float32),
        "W2": rng.standard_normal((1, 32), dtype=np.float32) * 0.1,
        "b2": np.zeros(1, np.float32),
    }
    r = kernel(**fake)
    print("kernel ran, out shape", r.shape)


# revision 26
# speedup vs baseline: 1.2008x; 1.2008x over previous
"""Trainium2 Bass kernel for nn_AudioModel (LSTM(40->64) -> last-h -> MLP head).

Strategy (8 NeuronCores, pure data parallel):
  - Each core processes a 1024-row batch shard; no collectives.
  - Device layout is fully transposed ("gate-major"): hidden units live on
    SBUF partitions, batch on the free dim. The per-core batch of 1024 is
    split into two halves A/B stacked on partitions (rows 0-63 = half A,
    rows 64-127 = half B) so every elementwise op runs on all 128 lanes.
  - Matmuls are block-diagonal over the A/B halves:
      gates_X[128, 512] = wx_X[82, 128].T @ ft_t[82, 512]   (x-side, + bias)
                        + wh_X[128, 128].T @ h_prev[128, 512] (h-side)
    where ft_t carries both halves' features plus constant-1 rows (bias).
  - All nonlinearities are reparameterized to sigmoid only:
      tanh(x) = 2*sigmoid(2x) - 1, folded into weight scaling and fused
      scalar_tensor_tensor ops; h is tracked as h' = h/2.
    So ScalarE does one [128, 2048] sigmoid over all 4 gates + one
    [128, 512] sigmoid for tanh(c) per step.
  - feats are transposed on the host into [T, 82, 512] per-core tiles so all
    device DMAs are clean contiguous loads.
"""

import os
import sys
from contextlib import ExitStack

import ml_dtypes
import numpy as np
_BF = np.dtype(np.float16)

for _p in ("/opt/trn_rl_repo",):
    if _p not in sys.path:
        sys.path.insert(0, _p)

import concourse.bass as bass
import concourse.mybir as mybir
from concourse import bacc
from concourse.bass_utils import run_bass_kernel_spmd
from concourse.tile import TileContext


def _install_ntff_hook():
    """Provide antenv.axon_hooks if the image lacks it, so trace=True works."""
    try:
        import antenv.axon_hooks  # noqa: F401

        return
    except ImportError:
        pass
    import contextlib
    import ctypes
    import types

    so_path = "/opt/axon/libaxon_pjrt.so"
    hook = None
    if os.path.exists(so_path):
        try:
            lib = ctypes.CDLL(so_path)
            if hasattr(lib, "axon_start_nrt_profile"):
                lib.axon_start_nrt_profile.argtypes = [
                    ctypes.POINTER(ctypes.c_int64),
                    ctypes.c_size_t,
                ]
                lib.axon_start_nrt_profile.restype = ctypes.c_int64
                lib.axon_stop_nrt_profile.argtypes = [ctypes.c_char_p]
                lib.axon_stop_nrt_profile.restype = ctypes.c_int64

                @contextlib.contextmanager
                def _hook(output_dir, device_ids):
                    import jax

                    jax.devices()
                    if device_ids:
                        ids = (ctypes.c_int64 * len(device_ids))(*device_ids)
                        rc = lib.axon_start_nrt_profile(ids, len(device_ids))
                    else:
                        rc = lib.axon_start_nrt_profile(None, 0)
                    if rc != 0:
                        raise RuntimeError(f"axon_start_nrt_profile rc={rc}")
                    try:
                        yield
                    finally:
                        n = lib.axon_stop_nrt_profile(str(output_dir).encode())
                        print(f"profile: {n} file(s) written to {output_dir}", file=sys.stderr)

                hook = _hook
        except OSError:
            hook = None

    mod = types.ModuleType("antenv.axon_hooks")
    mod._hook = hook
    mod.get_axon_ntff_profile_hook = lambda: mod._hook
    mod.set_axon_ntff_profile_hook = lambda h: setattr(mod, "_hook", h)
    sys.modules["antenv.axon_hooks"] = mod


_install_ntff_hook()


def _enable_ldw_opt():
    """walrus is invoked with --enable-ldw-opt=false; our inner loop reloads
    identical PE weights for back-to-back stream matmuls, so dedup helps."""
    from concourse import bass_utils as _bu

    if getattr(_bu, "_ldw_patch", False):
        return
    _orig = _bu.run_command

    def _patched(cmd, *a, **kw):
        return _orig(cmd, *a, **kw)

    _bu.run_command = _patched
    _bu._ldw_patch = True


# --enable-ldw-opt=true breaks walrus codegen (visitInstLdweights); keep off.

F32 = mybir.dt.float32
F32R = mybir.dt.float32r
FH = mybir.dt.float16
AF = mybir.ActivationFunctionType
OP = mybir.AluOpType

B, T, NI, H = 8192, 100, 40, 64
NCORES = 8
BL = B // NCORES  # 1024 rows per core
HB = BL // 2  # 512 = half-batch (free dim of all tiles)
KX = 2 * (NI + 1)  # 82 = A feats(40) + ones(1) + B feats(40) + ones(1)

LAST_RESULT = None
_NC_CACHE = {}


def _build_nc():
    nc = bacc.Bacc("TRN2", target_bir_lowering=False, debug=False)

    ft = nc.dram_tensor("ft", [T, KX, HB], FH, kind="ExternalInput")
    wx = nc.dram_tensor("wx", [KX, 512], FH, kind="ExternalInput")
    wh = nc.dram_tensor("wh", [128, 512], FH, kind="ExternalInput")
    w1 = nc.dram_tensor("w1", [128, 64], FH, kind="ExternalInput")
    b1 = nc.dram_tensor("b1", [64, 1], F32, kind="ExternalInput")
    w2 = nc.dram_tensor("w2", [64, 2], FH, kind="ExternalInput")
    b2 = nc.dram_tensor("b2", [2, 1], F32, kind="ExternalInput")
    out = nc.dram_tensor("out", [2, HB], F32, kind="ExternalOutput")

    with TileContext(nc) as tc, ExitStack() as ctx:
        const = ctx.enter_context(tc.tile_pool(name="const", bufs=1))
        ftp = ctx.enter_context(tc.tile_pool(name="ftp", bufs=12))
        gp = ctx.enter_context(tc.tile_pool(name="gp", bufs=1, space="PSUM"))
        sp = ctx.enter_context(tc.tile_pool(name="sp", bufs=3))
        dp = ctx.enter_context(tc.tile_pool(name="dp", bufs=4))
        hp = ctx.enter_context(tc.tile_pool(name="hp", bufs=3))

        wx_s = const.tile([KX, 512], FH)
        nc.sync.dma_start(wx_s[:], wx[:, :])
        wh_s = const.tile([128, 512], FH)
        nc.sync.dma_start(wh_s[:], wh[:, :])
        w1_s = const.tile([128, 64], FH)
        nc.sync.dma_start(w1_s[:], w1[:, :])
        b1_s = const.tile([64, 1], F32)
        nc.sync.dma_start(b1_s[:], b1[:, :])
        w2_s = const.tile([64, 2], FH)
        nc.sync.dma_start(w2_s[:], w2[:, :])
        b2_s = const.tile([2, 1], F32)
        nc.sync.dma_start(b2_s[:], b2[:, :])

        c2 = const.tile([128, HB], FH)  # cell state (fp32, in-place)
        h_final = const.tile([128, HB], FH)  # last step's h' for the head

        # Two phase-shifted streams over the free dim (cols 0:256 / 256:512)
        # so PE / ScalarE / VectorE overlap across the serial recurrence.
        NS = 2
        SW = HB // NS  # 256
        h_prev = [None] * NS  # h' = h/2; h0 == 0 so step 0 skips h-matmuls

        for t in range(T):
            ft_t = ftp.tile([KX, HB], FH)
            nc.sync.dma_start(ft_t[:], ft[t])

            # x-side matmuls for both streams first (no h dependency; adjacent
            # same-weight pairs dedupe their LDWEIGHTS under --enable-ldw-opt),
            # then per-stream h-side matmuls on the critical chain.
            # one full psum bank per gate per stream (8 banks total, bufs=1)
            # so accumulation groups never share a bank and x-side matmuls can
            # run ahead of the h-dependency without clearing sibling gates.
            gates_t = []
            for s in range(NS):
                gates_t.append(gp.tile([128, 4 * 512], F32, tag=f"g{s}", name=f"g{s}_{t}"))
            for X in range(4):
                for s in range(NS):
                    cs = slice(SW * s, SW * (s + 1))
                    nc.tensor.matmul(
                        gates_t[s][:, 512 * X : 512 * X + SW],
                        wx_s[:, 128 * X : 128 * (X + 1)],
                        ft_t[:, cs],
                        start=True,
                        stop=(h_prev[s] is None),
                    )
            for s in range(NS):
                if h_prev[s] is None:
                    continue
                for X in range(4):
                    nc.tensor.matmul(
                        gates_t[s][:, 512 * X : 512 * X + SW],
                        wh_s[:, 128 * X : 128 * (X + 1)],
                        h_prev[s],
                        start=False,
                        stop=True,
                    )

            for s in range(NS):
                cs = slice(SW * s, SW * (s + 1))
                gates = gates_t[s]
                S = sp.tile([128, 4 * SW], FH, tag=f"S{s}")
                # one sigmoid over all 4 banks; bank g holds sig(2*a_g)
                gv = gates[:, :].rearrange("p (g c) -> p g c", c=512)[:, :, 0:SW]
                sv = S[:, :].rearrange("p (g c) -> p g c", c=SW)
                nc.scalar.activation(sv, gv, AF.Sigmoid)
                sig_i = S[:, 0 * SW : 1 * SW]
                sig_f = S[:, 1 * SW : 2 * SW]
                sig_o = S[:, 2 * SW : 3 * SW]
                sig_g = S[:, 3 * SW : 4 * SW]
                c2s = c2[:, cs]

                # c2 holds c/2:  c/2 = (sig(2g)-0.5)*i + f*(c/2)_prev
                if t == 0:
                    nc.vector.scalar_tensor_tensor(c2s, sig_g, -0.5, sig_i, OP.add, OP.mult)
                else:
                    t1 = dp.tile([128, SW], FH, tag=f"t1{s}")
                    nc.vector.scalar_tensor_tensor(t1[:], sig_g, -0.5, sig_i, OP.add, OP.mult)
                    fm = dp.tile([128, SW], FH, tag=f"fm{s}")
                    nc.vector.tensor_mul(fm[:], sig_f, c2s)
                    nc.vector.tensor_add(c2s, t1[:], fm[:])
                # scv = tanh(2 * c/2) = tanh(c)
                scv = dp.tile([128, SW], FH, tag=f"scv{s}")
                nc.scalar.activation(scv[:], c2s, AF.Tanh, scale=2.0)
                # h = o * tanh(c)
                if t == T - 1:
                    h_new = h_final[:, cs]
                else:
                    h_new = hp.tile([128, SW], FH, name=f"hn{s}_{t}", tag=f"h{s}")[:]
                nc.vector.tensor_mul(h_new, scv[:], sig_o)
                h_prev[s] = h_new

        # classifier head: relu(2*W1 @ h' + b1) then W2 @ . + b2
        hid_ps = gp.tile([64, HB], F32, tag="g0")
        nc.tensor.matmul(hid_ps[:], w1_s[:], h_final[:], start=True, stop=True)
        hr = dp.tile([64, HB], FH, tag="hr")
        nc.scalar.activation(hr[:], hid_ps[:], AF.Relu, bias=b1_s[:])
        sc_ps = gp.tile([2, HB], F32, tag="g1")
        nc.tensor.matmul(sc_ps[:], w2_s[:], hr[:], start=True, stop=True)
        ov = dp.tile([2, HB], F32, tag="ov")
        nc.scalar.activation(ov[:], sc_ps[:], AF.Identity, bias=b2_s[:])
        nc.sync.dma_start(out[:, :], ov[:])

    nc.compile()
    return nc


def _get_nc():
    if "nc" not in _NC_CACHE:
        _NC_CACHE["nc"] = _build_nc()
    return _NC_CACHE["nc"]


def _prep_weights(inputs):
    W_ih = np.asarray(inputs["W_ih"], np.float32)  # [256, 40], gate order i,f,g,o
    W_hh = np.asarray(inputs["W_hh"], np.float32)  # [256, 64]
    bias = (np.asarray(inputs["b_ih"], np.float32) + np.asarray(inputs["b_hh"], np.float32))
    W1 = np.asarray(inputs["W1"], np.float32)  # [32, 64]
    b1 = np.asarray(inputs["b1"], np.float32)  # [32]
    W2 = np.asarray(inputs["W2"], np.float32)  # [1, 32]
    b2 = np.asarray(inputs["b2"], np.float32)  # [1]

    # device gate-bank order [i, f, o, g]; bank g carries 2x scale (sig(2x) trick)
    gate_order = [0, 1, 3, 2]
    gate_scale = [1.0, 1.0, 1.0, 2.0]
    wx = np.zeros((KX, 512), _BF)
    wh = np.zeros((128, 512), _BF)
    for X, gsel in enumerate(gate_order):
        sc = gate_scale[X]
        Wxe = (sc * W_ih[64 * gsel : 64 * (gsel + 1)]).astype(np.float32)  # [64, 40]
        Whe = (sc * W_hh[64 * gsel : 64 * (gsel + 1)]).astype(np.float32)  # [64, 64]
        be = (sc * bias[64 * gsel : 64 * (gsel + 1)]).astype(np.float32)  # [64]
        wx[0:NI, 128 * X : 128 * X + 64] = Wxe.T
        wx[NI, 128 * X : 128 * X + 64] = be
        wx[NI + 1 : 2 * NI + 1, 128 * X + 64 : 128 * X + 128] = Wxe.T
        wx[2 * NI + 1, 128 * X + 64 : 128 * X + 128] = be
        wh[0:64, 128 * X : 128 * X + 64] = Whe.T
        wh[64:128, 128 * X + 64 : 128 * X + 128] = Whe.T

    w1 = np.zeros((128, 64), _BF)
    w1[0:64, 0:32] = W1.T
    w1[64:128, 32:64] = W1.T
    b1v = np.concatenate([b1, b1]).reshape(64, 1).astype(np.float32)
    w2m = np.zeros((64, 2), _BF)
    w2m[0:32, 0] = W2[0]
    w2m[32:64, 1] = W2[0]
    b2v = np.array([[b2[0]], [b2[0]]], np.float32)
    return wx, wh, w1, b1v, w2m, b2v


def kernel(**inputs):
    global LAST_RESULT
    feats = np.asarray(inputs["feats"], np.float32)
    wx, wh, w1m, b1v, w2m, b2v = _prep_weights(inputs)

    in_maps = []
    for c in range(NCORES):
        shard = feats[c * BL : (c + 1) * BL]  # [1024, 100, 40]
        x = np.ascontiguousarray(shard.transpose(1, 2, 0))  # [100, 40, 1024]
        ftc = np.empty((T, KX, HB), _BF)
        ftc[:, 0:NI, :] = x[:, :, 0:HB]
        ftc[:, NI, :] = 1.0
        ftc[:, NI + 1 : 2 * NI + 1, :] = x[:, :, HB:]
        ftc[:, 2 * NI + 1, :] = 1.0
        in_maps.append(
            {"ft": ftc, "wx": wx, "wh": wh, "w1": w1m, "b1": b1v, "w2": w2m, "b2": b2v}
        )

    nc = _get_nc()
    trace = bool(os.environ.get("KERNEL_TRACE"))
    res = run_bass_kernel_spmd(nc, in_maps, core_ids=list(range(NCORES)), trace=trace)
    LAST_RESULT = res

    outs = np.empty((B, 1), np.float32)
    for c in range(NCORES):
        o = np.asarray(res.results[c]["out"])  # [2, 512]
        outs[c * BL : c * BL + HB, 0] = o[0]
        outs[c * BL + HB : (c + 1) * BL, 0] = o[1]
    return outs


if __name__ == "__main__":
    rng = np.random.default_rng(0)
    fake = {
        "feats": rng.standard_normal((B, T, NI), dtype=np.float32),
        "W_ih": rng.standard_normal((256, NI), dtype=np.float32) * 0.1,
        "W_hh": rng.standard_normal((256, H), dtype=np.float32) * 0.1,
        "b_ih": rng.standard_normal(256, dtype=np.float32) * 0.1,
        "b_hh": rng.standard_normal(256, dtype=np.float32) * 0.1,
        "W1": rng.standard_normal((32, H), dtype=np.float32) * 0.1,
        "b1": np.zeros(32, np.float32),
        "W2": rng.standard_normal((1, 32), dtype=np.float32) * 0.1,
        "b2": np.zeros(1, np.float32),
    }
    r = kernel(**fake)
    print("kernel ran, out shape", r.shape)
